# revision 34
# baseline (speedup 1.0000x reference)
"""Trainium2 Bass kernel for stacked per-position FC layer (Conv1d k=1 bank).

Computes out[b, o, i] = sum_c x[b, c, i] * W[i, o, c] + bias[i, o]
for x [64, 256, 2048], W [2048, 256, 256], bias [2048, 256] (fp32).

Strategy: shard positions (2048) across 8 NeuronCores (256 each) —
embarrassingly parallel, no collectives. Positions are processed in
PAIRS packed into the PE array via column tiling: position j's
stationary x-tile [c=128, b=64] sits in PE columns 0-63
(tile_position (0,0)), position j+1's in columns 64-127 ((0,64)).
Their matmuls run concurrently in the array and each LDWEIGHTS hides
under the other half's matmul. PSUM holds [2B=128, O=256] per pair;
bias is added with one K=2 matmul against a two-row indicator matrix.
Inputs stream as fp16 (fp32 accumulate in PSUM), output returns fp16
and is upcast on the host.

Host pre-permutes inputs to channel-major / position-middle layouts so
every device DMA is a [128-partition x multi-KB-contiguous-run] pattern:
  x -> [c, i, b]   W -> [c, i, o]   out <- [2b-half, i-pair, o]
"""

import numpy as np

import concourse.bacc as bacc
import concourse.bass as bass
import concourse.mybir as mybir
import concourse.tile as tile
from concourse.bass_utils import run_bass_kernel_spmd

N_CORES = 8
N_POS = 2048
P_LOC = N_POS // N_CORES  # 256 positions per core
C = 256  # contraction (c_in)
B = 64   # batch
O = 256  # c_out
KP = 128  # contraction tile (partition dim)
KT = C // KP  # 2 k-tiles

# Tunables
T = 32                       # positions per DMA tile (even)
MM_DT = mybir.dt.float16     # main matmul dtype
BIAS_DT = mybir.dt.float16   # bias matmul dtype
IO_DT = mybir.dt.float16     # dtype of declared DRAM params / SBUF tiles


def build_program(p_loc=P_LOC, t=T, mm_dt=MM_DT, bias_dt=BIAS_DT, io_dt=IO_DT):
    nc = bacc.Bacc("TRN2", target_bir_lowering=False, debug=False)
    fio = io_dt
    xt = nc.declare_dram_parameter("xt", [C, p_loc, B], fio, isOutput=False)
    wt = nc.declare_dram_parameter("wt", [C, p_loc, O], fio, isOutput=False)
    bt = nc.declare_dram_parameter("bt", [p_loc, O], fio, isOutput=False)
    ones_d = nc.declare_dram_parameter("ones2", [2, 2 * B], fio, isOutput=False)
    out = nc.declare_dram_parameter("out", [2 * B, p_loc // 2, O], io_dt,
                                    isOutput=True)

    n_tiles = p_loc // t
    tp = t // 2  # pairs per tile

    with tile.TileContext(nc) as tc:
        with (
            tc.tile_pool(name="wp", bufs=4 * KT) as w_pool,
            tc.tile_pool(name="xp", bufs=2 * KT) as x_pool,
            tc.tile_pool(name="bp", bufs=3) as b_pool,
            tc.tile_pool(name="op", bufs=4) as o_pool,
            tc.tile_pool(name="cp", bufs=1) as c_pool,
            tc.tile_pool(name="pp", bufs=6, space="PSUM") as ps_pool,
        ):
            ones = c_pool.tile([2, 2 * B], fio)

            # tile schedule: big tiles, but drain the tail in small chunks
            tiles = []
            pos = 0
            while pos < p_loc - t:
                tiles.append((pos, t))
                pos += t
            while pos < p_loc:
                tiles.append((pos, min(8, p_loc - pos)))
                pos += 8

            for it, (p0, tt) in enumerate(tiles):
                pr0 = p0 // 2
                ttp = tt // 2
                w_sb = []
                x_sb = []
                th = max(tt // 2, 1)  # W half-tile size (positions)
                for k in range(KT):
                    # balance bytes across the two HWDGE rings (SP + ACT)
                    w_eng = nc.sync if k == 0 else nc.scalar
                    x_eng = nc.scalar if k == 0 else nc.sync
                    halves = []
                    for hp0 in range(p0, p0 + tt, th):
                        hn = min(th, p0 + tt - hp0)
                        wk = w_pool.tile([KP, (t // 2) * O], fio, tag="w")
                        w_eng.dma_start(
                            out=wk[:, :hn * O],
                            in_=wt[k * KP:(k + 1) * KP, hp0:hp0 + hn, :],
                        )
                        halves.append(wk)
                    w_sb.append(halves)
                    xk = x_pool.tile([KP, t * B], fio, tag="x")
                    x_eng.dma_start(
                        out=xk[:, :tt * B],
                        in_=xt[k * KP:(k + 1) * KP, p0:p0 + tt, :],
                    )
                    x_sb.append(xk)
                # bias: [2, ttp*O]; partition 0 = even positions, 1 = odd
                bsb = b_pool.tile([2, (t // 2) * O], fio, tag="b")
                nc.sync.dma_start(
                    out=bsb[0:2, :ttp * O].rearrange(
                        "two (pr o) -> two pr o", pr=ttp),
                    in_=bt[p0:p0 + tt, :].rearrange("(pr two) o -> two pr o",
                                                    two=2),
                )
                if it == 0:
                    nc.sync.dma_start(out=ones[0:2, :], in_=ones_d[0:2, :])

                ob = o_pool.tile([2 * B, (t // 2) * O], io_dt, tag="ob")
                for pr in range(ttp):
                    j0 = 2 * pr
                    j1 = j0 + 1
                    ps = ps_pool.tile([2 * B, O], mybir.dt.float32)
                    h = j0 // th
                    r0 = j0 - h * th
                    r1 = r0 + 1
                    for k in range(KT):
                        wh = w_sb[k][h]
                        nc.tensor.matmul(
                            ps[0:B, :],
                            x_sb[k][:, j0 * B:(j0 + 1) * B].bitcast(mm_dt),
                            wh[:, r0 * O:(r0 + 1) * O].bitcast(mm_dt),
                            start=(k == 0),
                            stop=False,
                            tile_position=(0, 0),
                            skip_group_check=True,
                        )
                        nc.tensor.matmul(
                            ps[B:2 * B, :],
                            x_sb[k][:, j1 * B:(j1 + 1) * B].bitcast(mm_dt),
                            wh[:, r1 * O:(r1 + 1) * O].bitcast(mm_dt),
                            start=(k == 0),
                            stop=False,
                            tile_position=(0, B),
                            skip_group_check=True,
                        )
                    # bias for both halves: K=2 indicator matmul
                    nc.tensor.matmul(
                        ps[:, :],
                        ones[0:2, :].bitcast(bias_dt),
                        bsb[:, pr * O:(pr + 1) * O].bitcast(bias_dt),
                        start=False,
                        stop=True,
                        skip_group_check=True,
                    )
                    nc.vector.tensor_copy(ob[:, pr * O:(pr + 1) * O], ps[:, :])
                o_eng = nc.sync if it % 2 == 0 else nc.scalar
                o_eng.dma_start(
                    out=out[:, pr0:pr0 + ttp, :],
                    in_=ob[:, :ttp * O].rearrange("bb (pr o) -> bb pr o",
                                                  pr=ttp),
                )
    nc.compile()
    return nc


def _host_prep(x, W, b):
    """Permute inputs to device layouts; per-core contiguous slices.

    Returns xt8 [8, C, P_LOC, B], wt8 [8, C, P_LOC, O], bt8 [8, P_LOC, O].
    Uses jax on CPU when available (multithreaded transpose), else numpy.
    """
    np_dt = mybir.dt.np(IO_DT)
    try:
        import jax
        import jax.numpy as jnp
        cpu = jax.devices("cpu")[0]
        with jax.default_device(cpu):
            xj = jnp.asarray(np.asarray(x, dtype=np.float32))
            wj = jnp.asarray(np.asarray(W, dtype=np.float32))
            # x [B, C, 8*PL] -> [8, C, PL, B]
            xt8 = np.asarray(jnp.transpose(
                xj.reshape(B, C, N_CORES, P_LOC), (2, 1, 3, 0)).astype(np_dt))
            # W [8*PL, O, C] -> [8, C, PL, O]
            wt8 = np.asarray(jnp.transpose(
                wj.reshape(N_CORES, P_LOC, O, C), (0, 3, 1, 2)).astype(np_dt))
    except Exception:
        x = np.asarray(x, dtype=np.float32)
        W = np.asarray(W, dtype=np.float32)
        xt8 = np.ascontiguousarray(
            x.reshape(B, C, N_CORES, P_LOC).transpose(2, 1, 3, 0)).astype(np_dt)
        wt8 = np.ascontiguousarray(
            W.reshape(N_CORES, P_LOC, O, C).transpose(0, 3, 1, 2)).astype(np_dt)
    bt8 = np.ascontiguousarray(
        np.asarray(b, dtype=np.float32).reshape(N_CORES, P_LOC, O)).astype(np_dt)
    return xt8, wt8, bt8


def make_ones2():
    np_dt = mybir.dt.np(IO_DT)
    ones2 = np.zeros((2, 2 * B), np_dt)
    ones2[0, :B] = 1
    ones2[1, B:] = 1
    return ones2


def make_in_maps(x, W, b):
    xt8, wt8, bt8 = _host_prep(x, W, b)
    ones2 = make_ones2()
    return [{"xt": xt8[d], "wt": wt8[d], "bt": bt8[d], "ones2": ones2}
            for d in range(N_CORES)]


def run(in_maps, trace=False, **kwargs):
    nc = build_program()
    return run_bass_kernel_spmd(nc, in_maps, list(range(N_CORES)),
                                trace=trace, **kwargs)


def assemble_output(results):
    # results[d]["out"]: [2B, P_LOC//2, O]; partition half = even/odd position
    out = np.empty((B, O, N_POS), np.float32)
    for d in range(N_CORES):
        r = np.asarray(results[d]["out"], dtype=np.float32)
        r = r.reshape(2, B, P_LOC // 2, O)         # [half, b, pair, o]
        r = r.transpose(1, 3, 2, 0)                # [b, o, pair, half]
        out[:, :, d * P_LOC:(d + 1) * P_LOC] = r.reshape(B, O, P_LOC)
    return out


def kernel(x, W, b):
    in_maps = make_in_maps(x, W, b)
    res = run(in_maps)
    return assemble_output(res.results)
